# revision 3
# baseline (speedup 1.0000x reference)
"""Causal multi-head attention (B=2, S=2048, D=1024, H=16, d_k=64) on 8
Trainium2 NeuronCores.

Sharding: tensor-parallel over heads x data-parallel over batch.
Core (b*4 + c) computes batch b, heads 4c..4c+3 (a 256-wide d-slice):
  Q^T/K^T = W_slice @ x^T              (d-major, f32r)
  V       = x @ Wv_slice^T             (seq-major, f32r, + ones column)
  S^T     = K^T.T-free matmul          ([k,q] tiles, causal tiles only)
  P^T     = exp(S^T/8 + diag_mask)     (ACT, f32r out)
  A^T,l   = [V|1].T-style PV matmul    (accumulated in PSUM; row 64 = l)
  A^T    /= l                          (recip + gpsimd partition-broadcast)
  out_c   = A^T.T @ Wo_slice^T         (partial over the d-slice)
Host sums the 4 partial outputs per batch (the W_o row-parallel reduce).

All matmul inputs are float32r (TF32): full PE rate at N>=256 with
~1.5e-4 matmul relative error (measured on HW).
"""

import numpy as np

import concourse.bass as bass
import concourse.mybir as mybir
from concourse import bacc
from concourse.tile import TileContext
from concourse.bass_utils import run_bass_kernel_spmd

F32 = mybir.dt.float32
F32R = mybir.dt.float32r
Exp = mybir.ActivationFunctionType.Exp

B = 2
S = 2048
D = 1024
HEADS_PER_CORE = 4
DC = 64 * HEADS_PER_CORE  # 256: d-slice width per core
N_CORES = 8
NEG = -1.0e9


def _round_tf32(x: np.ndarray) -> np.ndarray:
    i = np.ascontiguousarray(x).view(np.uint32)
    return ((i + 0x1000) & 0xFFFFE000).view(np.float32)


def _build_mask() -> np.ndarray:
    """Additive mask for the 4-k-tile diagonal group of a 512-wide q-chunk.

    mask[k', tg*512 + q''] = 0 if 128*tg + k' <= q'' else NEG
    (tg = k-tile index within the group)."""
    kp = np.arange(128)[:, None]  # k'
    qq = np.arange(512)[None, :]  # q''
    blocks = [np.where(128 * tg + kp <= qq, 0.0, NEG) for tg in range(4)]
    return np.concatenate(blocks, axis=1).astype(np.float32)


def _build_nc():
    nc = bacc.Bacc("TRN2", target_bir_lowering=False)
    xT = nc.declare_dram_parameter("xT", [D, S], F32R, isOutput=False)
    wqT = nc.declare_dram_parameter("wqT", [D, DC], F32R, isOutput=False)
    wkT = nc.declare_dram_parameter("wkT", [D, DC], F32R, isOutput=False)
    wvT = nc.declare_dram_parameter("wvT", [D, DC], F32R, isOutput=False)
    woT = nc.declare_dram_parameter("woT", [DC, D], F32R, isOutput=False)
    maskd = nc.declare_dram_parameter("mask", [128, 2048], F32, isOutput=False)
    outd = nc.declare_dram_parameter("out", [S, D], F32, isOutput=True)

    with TileContext(nc) as tc:
        with (
            tc.tile_pool(name="persist", bufs=1) as pp,
            tc.tile_pool(name="vpool", bufs=1) as vp,
        ):
            # persistent tiles
            QT = pp.tile([64, 4 * S], F32R, name="QT")  # 4 heads concat
            KT = pp.tile([64, 4 * S], F32R, name="KT")
            woT_sb = [pp.tile([128, D], F32R, name=f"wo{t}") for t in range(2)]
            mask_sb = pp.tile([128, 2048], F32, name="mask")
            ones_sb = pp.tile([128, 1], F32, name="ones")
            V_sb = [vp.tile([128, 65 * 4], F32R, name=f"V{kt}") for kt in range(16)]

            nc.sync.dma_start(mask_sb[:], maskd[:])
            for t in range(2):
                nc.sync.dma_start(woT_sb[t][:], woT[128 * t : 128 * t + 128, :])
            nc.vector.memset(ones_sb[:], 1.0)

            # ---------------- Phase 1: projections ----------------
            with (
                tc.tile_pool(name="xw", bufs=1) as xw,
                tc.tile_pool(name="psQ", bufs=3, space="PSUM") as psQp,
                tc.tile_pool(name="psV", bufs=3, space="PSUM") as psVp,
            ):
                xT_sb = []
                wq_sb = []
                wk_sb = []
                wv_sb = []
                for dt in range(8):
                    r = slice(128 * dt, 128 * dt + 128)
                    xt = xw.tile([128, S], F32R, name=f"x{dt}")
                    nc.sync.dma_start(xt[:], xT[r, :])
                    xT_sb.append(xt)
                    for nm, dram, lst in (
                        ("q", wqT, wq_sb),
                        ("k", wkT, wk_sb),
                        ("v", wvT, wv_sb),
                    ):
                        wt = xw.tile([128, DC], F32R, name=f"w{nm}{dt}")
                        nc.sync.dma_start(wt[:], dram[r, :])
                        lst.append(wt)

                # Q^T, K^T: per head-pair t, per q-chunk j
                for w_sb, dst in ((wq_sb, QT), (wk_sb, KT)):
                    for t in range(2):
                        for j in range(4):
                            ps = psQp.tile([128, 512], F32, name="psq", tag="psq")
                            for dt in range(8):
                                nc.tensor.matmul(
                                    ps[:],
                                    w_sb[dt][:, 128 * t : 128 * t + 128],
                                    xT_sb[dt][:, 512 * j : 512 * j + 512],
                                    start=(dt == 0),
                                    stop=(dt == 7),
                                )
                            for hh in range(2):  # split stacked heads
                                h = 2 * t + hh
                                nc.vector.tensor_copy(
                                    dst[:, S * h + 512 * j : S * h + 512 * j + 512],
                                    ps[64 * hh : 64 * hh + 64, :],
                                )

                # V (seq-major) + ones columns
                for kt in range(16):
                    ps = psVp.tile([128, DC], F32, name="psv", tag="psv")
                    for dt in range(8):
                        nc.tensor.matmul(
                            ps[:],
                            xT_sb[dt][:, 128 * kt : 128 * kt + 128],
                            wv_sb[dt][:],
                            start=(dt == 0),
                            stop=(dt == 7),
                        )
                    for h in range(4):
                        nc.vector.tensor_copy(
                            V_sb[kt][:, 65 * h : 65 * h + 64],
                            ps[:, 64 * h : 64 * h + 64],
                        )
                        nc.vector.tensor_copy(
                            V_sb[kt][:, 65 * h + 64 : 65 * h + 65], ones_sb[:]
                        )

            # ---------------- Phase 2+3: attention + out-proj ----------------
            with (
                tc.tile_pool(name="atp", bufs=1) as atp,
                tc.tile_pool(name="psS", bufs=2, space="PSUM") as psSp,
                tc.tile_pool(name="psA", bufs=2, space="PSUM") as psAp,
                tc.tile_pool(name="psO", bufs=2, space="PSUM") as psOp,
                tc.tile_pool(name="pt", bufs=3) as ptp,
                tc.tile_pool(name="norm", bufs=3) as np_,
                tc.tile_pool(name="osb", bufs=3) as op_,
            ):
                AT = [atp.tile([128, S], F32R, name=f"AT{t}") for t in range(2)]
                for j in range(4):  # q-chunk of 512
                    for h in range(4):
                        psA = psAp.tile([65, 512], F32, name="psa", tag="psa")
                        last_kt = 4 * j + 3
                        for p in range(2 * j + 2):  # kt pair
                            psS = psSp.tile([128, 1024], F32, name="pss", tag="pss")
                            for t2 in range(2):
                                kt = 2 * p + t2
                                nc.tensor.matmul(
                                    psS[:, 512 * t2 : 512 * t2 + 512],
                                    KT[:, S * h + 128 * kt : S * h + 128 * kt + 128],
                                    QT[:, S * h + 512 * j : S * h + 512 * j + 512],
                                    start=True,
                                    stop=True,
                                )
                            if p >= 2 * j:  # diagonal pair: causal mask
                                off = 1024 * (p - 2 * j)
                                nc.vector.tensor_add(
                                    psS[:], psS[:], mask_sb[:, off : off + 1024]
                                )
                            pt = ptp.tile([128, 1024], F32R, name="pt", tag="pt")
                            nc.scalar.activation(pt[:], psS[:], Exp, scale=0.125)
                            for t2 in range(2):
                                kt = 2 * p + t2
                                nc.tensor.matmul(
                                    psA[:],
                                    V_sb[kt][:, 65 * h : 65 * h + 65],
                                    pt[:, 512 * t2 : 512 * t2 + 512],
                                    start=(kt == 0),
                                    stop=(kt == last_kt),
                                )
                        # normalize: A^T[d, q] /= l[q]
                        lr = np_.tile([1, 512], F32, name="lr", tag="lr")
                        nc.vector.reciprocal(lr[:], psA[64:65, :])
                        rb = np_.tile([128, 512], F32, name="rb", tag="rb")
                        nc.gpsimd.partition_broadcast(rb[:], lr[:])
                        t, hh = divmod(h, 2)
                        po = 64 * hh
                        nc.vector.tensor_mul(
                            AT[t][po : po + 64, 512 * j : 512 * j + 512],
                            psA[0:64, :],
                            rb[po : po + 64, :],
                        )
                    # out-proj for the 4 q-tiles of this chunk
                    for qt in range(4 * j, 4 * j + 4):
                        for mc in range(2):
                            psO = psOp.tile([128, 512], F32, name="pso", tag="pso")
                            for t in range(2):
                                nc.tensor.matmul(
                                    psO[:],
                                    AT[t][:, 128 * qt : 128 * qt + 128],
                                    woT_sb[t][:, 512 * mc : 512 * mc + 512],
                                    start=(t == 0),
                                    stop=(t == 1),
                                )
                            ot = op_.tile([128, 512], F32, name="ot", tag="ot")
                            nc.scalar.copy(ot[:], psO[:])
                            nc.sync.dma_start(
                                outd[
                                    128 * qt : 128 * qt + 128,
                                    512 * mc : 512 * mc + 512,
                                ],
                                ot[:],
                            )

    nc.finalize()
    return nc


_NC = None


def _get_nc():
    global _NC
    if _NC is None:
        _NC = _build_nc()
    return _NC


def kernel(x, W_q, W_k, W_v, W_o):
    nc = _get_nc()
    mask = _build_mask()
    in_maps = []
    xTs = [_round_tf32(x[b].T) for b in range(B)]
    for core in range(N_CORES):
        b, c = divmod(core, 4)
        sl = slice(DC * c, DC * c + DC)
        in_maps.append(
            {
                "xT": xTs[b],
                "wqT": _round_tf32(W_q[sl, :].T),
                "wkT": _round_tf32(W_k[sl, :].T),
                "wvT": _round_tf32(W_v[sl, :].T),
                "woT": _round_tf32(W_o[:, sl].T),
                "mask": mask,
            }
        )
    res = run_bass_kernel_spmd(nc, in_maps, list(range(N_CORES)))
    outs = [res.results[i]["out"] for i in range(N_CORES)]
    full = np.stack(
        [outs[0] + outs[1] + outs[2] + outs[3], outs[4] + outs[5] + outs[6] + outs[7]]
    )
    return full.astype(np.float32)
